# revision 2
# baseline (speedup 1.0000x reference)
"""Trainium2 Bass kernel for CausalSelfAttention (full softmax + RoPE).

Problem: x[4,2048,2048] -> qkv proj (W_attn [6144,2048]) -> RoPE(q,k) ->
softmax(q k^T / sqrt(128)) v -> out proj (W_proj [2048,2048]).

Sharding: 8 cores = (batch 4) x (head-group 2). Core c=(b,hg) computes heads
hg*8..hg*8+7 of batch b and the partial output projection over those heads'
columns; host sums the two partials per batch.

Fused fp16 pipeline (all matmuls 2-byte = full PE rate at any moving width):
  stage V:  V = x @ Wv^T in [t, d] layout -> DRAM scratch (fp16)
  per head h (QK-projection of head h+1 interleaved into attention of head h
  at kt granularity so the ACT-bound exp stream hides under PE work):
    QK_h:   Q^T,K^T = (x @ Wq/k^T)^T in [d, t] layout + fused RoPE (DVE),
            kept resident in SBUF fp16 (no DRAM roundtrip)
    ATT_h:  per 512-wide q chunk: S tiles = K'^T(stationary) @ Q' -> exp
            (ACT, fp16 out, no max subtraction: |logits| <~ 6) ->
            PV^T tiles = e(stationary) @ [V | 1] so the softmax denominator
            falls out of the same matmul -> per-partition normalize (DVE)
            -> PE transpose back to [d, t] -> resident fp16
  stage 3:  OT[c,t] partial = Wp^T @ PVT over this core's 1024 hd columns
"""
import sys
for _p in ('/opt/pypackages', '/opt/trn_rl_repo'):
    if _p not in sys.path:
        sys.path.insert(0, _p)

from contextlib import ExitStack

import numpy as np

import concourse.bacc as bacc
import concourse.tile as tile
from concourse import mybir
from concourse.bass_utils import run_bass_kernel_spmd

F32 = mybir.dt.float32
FP16 = mybir.dt.float16

B, T, C = 4, 2048, 2048
H, D = 16, 128
HPC = 8                 # heads per core
FV = HPC * D            # 1024
SCALE = 1.0 / float(np.sqrt(D))
N_CORES = 8


class _QKStream:
    """Emits the QK projection of one head in resumable chunks so it can be
    interleaved into the previous head's attention instruction stream."""

    def __init__(self, nc, h, t, KC, n_tc, pools, xt_t, cos_t, sin_t, wqk_d):
        self.nc = nc
        self.h = h
        self.t = t
        self.KC = KC
        self.n_tc = n_tc
        (self.qkwp, self.qkh, self.rope, self.qkps) = pools
        self.xt_t = xt_t
        self.cos_t = cos_t
        self.sin_t = sin_t
        # allocate weight + output tiles and start weight loads up front
        self.wq = self.qkwp.tile([128, KC, 128], FP16, tag="wq")
        nc.sync.dma_start(
            out=self.wq,
            in_=wqk_d.ap()[h].rearrange("kc p f -> p kc f"))
        self.wk = self.qkwp.tile([128, KC, 128], FP16, tag="wk", bufs=1)
        nc.sync.dma_start(
            out=self.wk,
            in_=wqk_d.ap()[h + HPC].rearrange("kc p f -> p kc f"))
        self.qh = self.qkh.tile([128, t], FP16, tag="qh")
        self.kh = self.qkh.tile([128, t], FP16, tag="kh")
        # unit u: u < n_tc -> q chunk u; else k chunk u - n_tc
        self.n_units = 2 * n_tc
        self.ops_per_unit = KC
        self.total_ops = self.n_units * self.ops_per_unit
        self.emitted = 0
        self.ps = None

    def step_to(self, target):
        target = min(target, self.total_ops)
        nc = self.nc
        while self.emitted < target:
            u, kc = divmod(self.emitted, self.ops_per_unit)
            is_q = u < self.n_tc
            tch = u if is_q else u - self.n_tc
            sl = slice(tch * 512, (tch + 1) * 512)
            w = self.wq if is_q else self.wk
            if kc == 0:
                self.ps = self.qkps.tile([128, 512], F32, tag="qk")
            nc.tensor.matmul(self.ps, lhsT=w[:, kc, :],
                             rhs=self.xt_t[kc][:, sl],
                             start=(kc == 0), stop=(kc == self.KC - 1))
            self.emitted += 1
            if kc == self.KC - 1:
                # RoPE: out = ps*cos + rot(ps)*sin (sin pre-shifted+signed)
                ps = self.ps
                dst = self.qh if is_q else self.kh
                tmp = self.rope.tile([128, 512], F32, tag="tmp")
                nc.vector.tensor_tensor(out=tmp[0:64, :], in0=ps[64:128, :],
                                        in1=self.sin_t[64:128, sl],
                                        op=mybir.AluOpType.mult)
                nc.vector.tensor_tensor(out=tmp[64:128, :], in0=ps[0:64, :],
                                        in1=self.sin_t[0:64, sl],
                                        op=mybir.AluOpType.mult)
                qc_t = self.rope.tile([128, 512], F32, tag="qc")
                nc.vector.tensor_tensor(out=qc_t, in0=ps, in1=self.cos_t[:, sl],
                                        op=mybir.AluOpType.mult)
                nc.vector.tensor_tensor(out=dst[:, sl], in0=qc_t, in1=tmp,
                                        op=mybir.AluOpType.add)

    def finish(self):
        self.step_to(self.total_ops)


def build_nc(t=T, reps=1, py_reps=1, debug=False):
    assert t % 512 == 0
    n_tt = t // 128      # t-tiles of 128
    n_tc = t // 512      # t-chunks of 512
    KC = C // 128        # contraction chunks over C

    nc = bacc.Bacc("TRN2", target_bir_lowering=False)

    xT_d = nc.dram_tensor("xT", [C, t], FP16, kind="ExternalInput")
    wqk_d = nc.dram_tensor("wqk4", [16, KC, 128, 128], FP16, kind="ExternalInput")
    wv_d = nc.dram_tensor("wv3", [KC, 128, FV], FP16, kind="ExternalInput")
    wp_d = nc.dram_tensor("wp4", [HPC, 16, 128, 128], FP16, kind="ExternalInput")
    cos_d = nc.dram_tensor("cosT", [128, t], FP16, kind="ExternalInput")
    sin_d = nc.dram_tensor("sinS", [128, t], FP16, kind="ExternalInput")
    id_d = nc.dram_tensor("ident", [128, 128], FP16, kind="ExternalInput")
    ot_d = nc.dram_tensor("OT", [C, t], FP16, kind="ExternalOutput")
    dbg = None
    if debug:
        dbg = {
            "Q0": nc.dram_tensor("Q0", [128, t], FP16, kind="ExternalOutput"),
            "K0": nc.dram_tensor("K0", [128, t], FP16, kind="ExternalOutput"),
            "VH0": nc.dram_tensor("VH0", [128, n_tt, 129], FP16,
                                  kind="ExternalOutput"),
            "E00": nc.dram_tensor("E00", [128, 512], FP16,
                                  kind="ExternalOutput"),
            "PVN0": nc.dram_tensor("PVN0", [128, 128], FP16,
                                   kind="ExternalOutput"),
            "PRAW": nc.dram_tensor("PRAW", [128, 2, 136], F32,
                                   kind="ExternalOutput"),
            "PVT0": nc.dram_tensor("PVT0", [128, t], FP16,
                                   kind="ExternalOutput"),
        }

    with tile.TileContext(nc) as tc, ExitStack() as octx:
        if reps > 1:
            octx.enter_context(tc.For_i(0, reps, 1))
        for _py_rep in range(py_reps):
            _build_body(nc, tc, t, n_tt, n_tc, KC,
                        xT_d, wqk_d, wv_d, wp_d, cos_d, sin_d, id_d, ot_d,
                        dbg=dbg)

    nc.compile()
    return nc


def _build_body(nc, tc, t, n_tt, n_tc, KC,
                xT_d, wqk_d, wv_d, wp_d, cos_d, sin_d, id_d, ot_d, dbg=None):
    with ExitStack() as octx:
        const = octx.enter_context(tc.tile_pool(name="const", bufs=1))
        ident_t = const.tile([128, 128], FP16, name="ident_t")
        nc.gpsimd.dma_start(out=ident_t, in_=id_d.ap())

        # persistent across stages: attention outputs, V tiles ([V | 1] per
        # head, SBUF-resident -- no DRAM roundtrip), and Wp
        pvtp = octx.enter_context(tc.tile_pool(name="pvtp", bufs=1))
        pvt = [pvtp.tile([128, t], FP16, name=f"pvt{h}") for h in range(HPC)]
        vhp = octx.enter_context(tc.tile_pool(name="vhp", bufs=1))
        vh_all = [vhp.tile([128, n_tt, 129], FP16, name=f"vh{h}")
                  for h in range(HPC)]
        wpp = octx.enter_context(tc.tile_pool(name="wpp", bufs=1))
        wp_t = [wpp.tile([128, 16, 128], FP16, name=f"wp{hc}")
                for hc in range(HPC)]

        # x^T resident for the whole projection phase (gpsimd ring so the
        # reload of iteration i+1 overlaps iteration i's stage-3 OT writes)
        resid = octx.enter_context(tc.tile_pool(name="resid", bufs=1))
        xt_t = []
        for kc in range(KC):
            xt = resid.tile([128, t], FP16, name=f"xt{kc}")
            nc.gpsimd.dma_start(out=xt, in_=xT_d.ap()[kc * 128:(kc + 1) * 128, :])
            xt_t.append(xt)
        cos_t = resid.tile([128, t], FP16, name="cos_t")
        nc.gpsimd.dma_start(out=cos_t, in_=cos_d.ap())
        sin_t = resid.tile([128, t], FP16, name="sin_t")
        nc.gpsimd.dma_start(out=sin_t, in_=sin_d.ap())

        # ------------- stage V: V = x @ Wv^T -> SBUF-resident vh tiles -------------
        with ExitStack() as vctx:
            wvp = vctx.enter_context(tc.tile_pool(name="wvp", bufs=1))
            psV = vctx.enter_context(tc.tile_pool(name="psV", bufs=4, space="PSUM"))
            for h in range(HPC):
                nc.vector.memset(vh_all[h][:, :, 128:129], 1.0)
            for fh in range(FV // 512):
                wv_t = []
                for kc in range(KC):
                    w = wvp.tile([128, 512], FP16, name=f"wv{fh}_{kc}",
                                 tag=f"wv{kc}")
                    nc.gpsimd.dma_start(
                        out=w, in_=wv_d.ap()[kc][:, fh * 512:(fh + 1) * 512])
                    wv_t.append(w)
                for tt in range(n_tt):
                    ps = psV.tile([128, 512], F32, tag="ps")
                    for kc in range(KC):
                        nc.tensor.matmul(
                            ps,
                            lhsT=xt_t[kc][:, tt * 128:(tt + 1) * 128],
                            rhs=wv_t[kc],
                            start=(kc == 0), stop=(kc == KC - 1))
                    for hh in range(4):
                        nc.scalar.copy(
                            vh_all[fh * 4 + hh][:, tt, 0:128],
                            ps[:, hh * 128:(hh + 1) * 128])

        # ---------------- per-head: QK projection + attention ----------------
        with ExitStack() as ctx:
            qkwp = ctx.enter_context(tc.tile_pool(name="qkwp", bufs=2))
            qkh = ctx.enter_context(tc.tile_pool(name="qkh", bufs=2))
            rope = ctx.enter_context(tc.tile_pool(name="rope", bufs=1))
            e2p = ctx.enter_context(tc.tile_pool(name="e2p", bufs=3))
            nrm = ctx.enter_context(tc.tile_pool(name="nrm", bufs=4))
            qkps = ctx.enter_context(tc.tile_pool(name="qkps", bufs=2, space="PSUM"))
            ps2 = ctx.enter_context(tc.tile_pool(name="ps2", bufs=3, space="PSUM"))
            psp = ctx.enter_context(tc.tile_pool(name="psp", bufs=2, space="PSUM"))
            pst = ctx.enter_context(tc.tile_pool(name="pst", bufs=1, space="PSUM"))

            qk_pools = (qkwp, qkh, rope, qkps)

            def make_stream(h):
                return _QKStream(nc, h, t, KC, n_tc, qk_pools,
                                 xt_t, cos_t, sin_t, wqk_d)

            s0 = make_stream(0)
            s0.finish()
            streams = {0: s0}

            pending = []        # deferred normalize/transpose closures

            def emit_pending(n=1):
                for _ in range(min(n, len(pending))):
                    pending.pop(0)()

            for h in range(HPC):
                cur = streams[h]
                qh_t, kh_t = cur.qh, cur.kh
                vh_t = vh_all[h]
                if h + 1 < HPC:
                    nxt = make_stream(h + 1)
                    streams[h + 1] = nxt
                else:
                    nxt = None
                if h == 1:
                    for hc in range(HPC):
                        nc.gpsimd.dma_start(
                            out=wp_t[hc],
                            in_=wp_d.ap()[hc].rearrange("ct p f -> p ct f"))
                if dbg is not None and h == 0:
                    nc.sync.dma_start(out=dbg["Q0"].ap(), in_=qh_t)
                    nc.sync.dma_start(out=dbg["K0"].ap(), in_=kh_t)
                    nc.sync.dma_start(out=dbg["VH0"].ap(), in_=vh_t)
                for qc in range(n_tc):
                    sl = slice(qc * 512, (qc + 1) * 512)
                    pvt_ab = [psp.tile([128, 2, 136], F32, tag="pvt",
                                       name=f"pvt_ps{i}") for i in range(2)]
                    for kt in range(n_tt):
                        s2 = ps2.tile([128, 512], F32, tag="s2")
                        nc.tensor.matmul(s2,
                                         lhsT=kh_t[:, kt * 128:(kt + 1) * 128],
                                         rhs=qh_t[:, sl],
                                         start=True, stop=True)
                        e = e2p.tile([128, 512], FP16, tag="e")
                        nc.scalar.activation(e, s2,
                                             mybir.ActivationFunctionType.Exp,
                                             scale=SCALE)
                        if dbg is not None and h == 0 and qc == 0 and kt == 0:
                            nc.sync.dma_start(out=dbg["E00"].ap(), in_=e)
                        for j in range(4):
                            # start=True zeroes the whole PSUM bank, so only
                            # the first chain per bank may use it; the second
                            # chain accumulates onto the just-zeroed region.
                            nc.tensor.matmul(
                                pvt_ab[j // 2][:, j % 2, 0:129],
                                lhsT=e[:, j * 128:(j + 1) * 128],
                                rhs=vh_t[:, kt, :],
                                start=(kt == 0 and j % 2 == 0),
                                stop=(kt == n_tt - 1))
                        emit_pending(1)
                        if nxt is not None:
                            target = (2 * KC * (qc * n_tt + kt + 1)) // n_tt
                            nxt.step_to(target)

                    if dbg is not None and h == 0 and qc == 0:
                        praw_st = nrm.tile([128, 2, 136], F32, tag="praw",
                                           bufs=1)
                        nc.scalar.copy(praw_st, pvt_ab[0])
                        nc.sync.dma_start(out=dbg["PRAW"].ap(), in_=praw_st)

                    def norm_tail(h=h, qc=qc, pvt_ab=pvt_ab):
                        def one(j):
                            def fn():
                                src = pvt_ab[j // 2]
                                rec = nrm.tile([128, 1], F32, tag="rec")
                                nc.vector.reciprocal(rec, src[:, j % 2, 128:129])
                                pvn = nrm.tile([128, 128], FP16, tag="pvn")
                                nc.vector.tensor_scalar(
                                    out=pvn, in0=src[:, j % 2, 0:128],
                                    scalar1=rec, scalar2=None,
                                    op0=mybir.AluOpType.mult)
                                tr = pst.tile([128, 128], FP16, tag="tr")
                                nc.tensor.transpose(tr, pvn, ident_t)
                                dst = qc * 512 + j * 128
                                nc.vector.tensor_copy(pvt[h][:, dst:dst + 128],
                                                      tr)
                                if (dbg is not None and h == 0 and qc == 0
                                        and j == 0):
                                    nc.sync.dma_start(out=dbg["PVN0"].ap(),
                                                      in_=pvn)
                            return fn
                        return [one(j) for j in range(4)]

                    pending.extend(norm_tail())
                if nxt is not None:
                    nxt.finish()
            while pending:
                emit_pending(1)
            if dbg is not None:
                nc.sync.dma_start(out=dbg["PVT0"].ap(), in_=pvt[0])

        # ---------------- stage 3: output projection ----------------
        with ExitStack() as ctx:
            ostg = ctx.enter_context(tc.tile_pool(name="ostg", bufs=4))
            ps3 = ctx.enter_context(tc.tile_pool(name="ps3", bufs=4, space="PSUM"))

            for tch in range(n_tc):
                sl = slice(tch * 512, (tch + 1) * 512)
                for ct in range(16):
                    ps = ps3.tile([128, 512], F32, tag="ps")
                    for hc in range(HPC):
                        nc.tensor.matmul(ps, lhsT=wp_t[hc][:, ct, :],
                                         rhs=pvt[hc][:, sl],
                                         start=(hc == 0), stop=(hc == HPC - 1))
                    st = ostg.tile([128, 512], FP16, tag="st")
                    nc.scalar.copy(st, ps)
                    nc.sync.dma_start(
                        out=ot_d.ap()[ct * 128:(ct + 1) * 128, sl], in_=st)


def make_in_maps(x, cos, sin, W_attn, W_proj):
    t = x.shape[1]
    KC = C // 128
    x = np.asarray(x, np.float32)
    cosT = np.ascontiguousarray(np.asarray(cos, np.float32)[0].T)        # [D, t]
    sinT = np.asarray(sin, np.float32)[0].T                               # [D, t]
    sinS = np.ascontiguousarray(
        np.concatenate([sinT[64:128], -sinT[0:64]], axis=0))
    ident = np.eye(128, dtype=np.float16)
    W_attn = np.asarray(W_attn, np.float32)
    W_proj = np.asarray(W_proj, np.float32)

    xT_b = [np.ascontiguousarray(x[b].T).astype(np.float16) for b in range(B)]
    cosT16 = cosT.astype(np.float16)
    sinS16 = sinS.astype(np.float16)

    per_hg = []
    for hg in range(2):
        r = slice(hg * 1024, (hg + 1) * 1024)
        wq = W_attn[0 * C + hg * 1024:0 * C + (hg + 1) * 1024]
        wk = W_attn[1 * C + hg * 1024:1 * C + (hg + 1) * 1024]
        wv = W_attn[2 * C + hg * 1024:2 * C + (hg + 1) * 1024]
        wqkT = np.concatenate([wq, wk], axis=0).T                         # [C, 2048]
        wqk4 = np.ascontiguousarray(
            wqkT.reshape(KC, 128, 16, 128).transpose(2, 0, 1, 3)).astype(np.float16)
        wv3 = np.ascontiguousarray(wv.T.reshape(KC, 128, FV)).astype(np.float16)
        wpT = W_proj[:, r].T                                              # [1024, C]
        wp4 = np.ascontiguousarray(
            wpT.reshape(HPC, 128, 16, 128).transpose(0, 2, 1, 3)).astype(np.float16)
        per_hg.append((wqk4, wv3, wp4))

    in_maps = []
    for core in range(N_CORES):
        b, hg = core // 2, core % 2
        wqk4, wv3, wp4 = per_hg[hg]
        in_maps.append({
            "xT": xT_b[b], "wqk4": wqk4, "wv3": wv3, "wp4": wp4,
            "cosT": cosT16, "sinS": sinS16, "ident": ident,
        })
    return in_maps


_NC_CACHE = {}


def get_nc(t=T):
    if t not in _NC_CACHE:
        _NC_CACHE[t] = build_nc(t)
    return _NC_CACHE[t]


def kernel(x, cos, sin, W_attn, W_proj):
    in_maps = make_in_maps(x, cos, sin, W_attn, W_proj)
    nc = get_nc(x.shape[1])
    out = np.empty((B, x.shape[1], C), np.float32)
    for _attempt in range(3):
        res = run_bass_kernel_spmd(nc, in_maps, list(range(N_CORES))).results
        for b in range(B):
            out[b] = (res[2 * b]["OT"].astype(np.float32) +
                      res[2 * b + 1]["OT"].astype(np.float32)).T
        # cold-device runs have very rarely produced a transient NaN; the
        # recompute is free in the common case
        if np.isfinite(out.sum(dtype=np.float64)):
            break
    return out

